# revision 25
# baseline (speedup 1.0000x reference)
"""Trainium2 Bass kernel for the sparse_attention nn.Module problem.

Strategy: data-parallel over the MSA-row dim S (S=128 -> 16 rows per core,
8 cores). All projection weights + pair bias replicated; activations and
mask sharded with S. No collectives.

Per-core dataflow (scheme C2 -- fully transposed attention, tile_position
packed matmuls, mask folded into v / Z so exp needs no bias, row-PAIRED
projections so the shared weights stream N=512, software-pipelined
emission: projections of pair p interleave with attention of pair p-1):
  qT/kT/gT = W @ [x_s0^T | x_s1^T]   (paired N=512 matmuls, PSUM f32,
                                      DVE evict fp16 / ACT tanh for gate;
                                      bg folded into the ACT bias)
  v'_s     = (kv_x @ Wv^T) * exp(mask)[k]    (mask folded into v rows)
  sT_h     = kT_h^T @ qT_h    (4-way ROW-packed tile_position=(32hh,0);
                               concurrent MMs drain into 4 distinct PSUM
                               banks across two 2-bank tiles scA/scB)
  expS     = exp(sT)          (no bias -> [128,1024] ACT ops)
  A        = expS * exp(pair) (DVE bf16; one chunk per row on GpSimd)
  oT_h     = v'_h^T @ A_h     (4-way COL-packed tile_position=(0,32hh))
  Zbc_h    = em^T @ A_h       (same col-packing, lhsT = exp(mask) x32
                               -> Z_h[q] lane-aligned with oT_h)
  og       = (tanh((gT+bg)/2)+1) * oT / Zbc   (0.5 folded into Wo)
  out      = og^T @ (0.5*Wo)^T + bo           (bo added in the eviction)
"""

import os
import numpy as np
import ml_dtypes

B, S, Q, C = 1, 128, 256, 256
H, DH = 8, 32
TOT = H * DH
N_CORES = 8
S_LOC = S // N_CORES  # 16

_CACHE = {}


def _build_program(s_loc):
    import concourse.bacc as bacc
    import concourse.mybir as mybir
    from concourse import tile

    dt = mybir.dt
    f32, bf16, f16 = dt.float32, dt.bfloat16, dt.float16
    AF = mybir.ActivationFunctionType
    ALU = mybir.AluOpType
    use_div = os.environ.get("KDIV", "recip") == "div"
    gp_mul = int(os.environ.get("KGP", "0"))  # A-mul chunks on GpSimd /row
    nsplit = int(os.environ.get("KSPLIT", "2"))  # hg >= nsplit: exp*mul path

    npair = s_loc // 2

    nc = bacc.Bacc("TRN2", target_bir_lowering=False, debug=False,
                   num_devices=N_CORES)

    x_d = nc.dram_tensor("x", [s_loc, 128, 1024], f16, kind="ExternalInput").ap()
    wq_d = nc.dram_tensor("wq", [128, 512], f16, kind="ExternalInput").ap()
    wk_d = nc.dram_tensor("wk", [128, 512], f16, kind="ExternalInput").ap()
    wv_d = nc.dram_tensor("wv", [128, 512], f16, kind="ExternalInput").ap()
    wg_d = nc.dram_tensor("wg", [128, 512], f16, kind="ExternalInput").ap()
    wo_d = nc.dram_tensor("wo", [128, 512], f16, kind="ExternalInput").ap()
    expb_d = nc.dram_tensor("expb", [128, 4096], bf16, kind="ExternalInput").ap()
    em_d = nc.dram_tensor("em", [128, s_loc * 64], bf16, kind="ExternalInput").ap()
    bgc_d = nc.dram_tensor("bgc", [128, 2], f32, kind="ExternalInput").ap()
    id_d = nc.dram_tensor("ident", [128, 128], bf16, kind="ExternalInput").ap()
    out_d = nc.dram_tensor("out", [s_loc, 128, 512], f32, kind="ExternalOutput").ap()

    with tile.TileContext(nc) as tc:
        with (
            tc.tile_pool(name="const", bufs=1) as cp,
            tc.tile_pool(name="work", bufs=4) as wp,
            tc.tile_pool(name="work4", bufs=6) as wp4,
            tc.tile_pool(name="pp", bufs=2, space="PSUM") as pp,
            tc.tile_pool(name="sca", bufs=2, space="PSUM") as pscA,
        ):
            # ---- resident constants ----
            wq_t = cp.tile([128, 512], f16, tag="wq")
            wk_t = cp.tile([128, 512], f16, tag="wk")
            wv_t = cp.tile([128, 512], f16, tag="wv")
            wg_t = cp.tile([128, 512], f16, tag="wg")
            wo_t = cp.tile([128, 512], f16, tag="wo")
            expb_t = cp.tile([128, 4096], bf16, tag="expb")
            em_t = cp.tile([128, s_loc * 64], bf16, tag="em")
            bgc_t = cp.tile([128, 2], f32, tag="bgc")
            id_t = cp.tile([128, 128], bf16, tag="ident")

            # spread constant loads across queues so the x/proj stream
            # on the sync queue is not stuck behind the 1MB pair tensor
            nc.sync.dma_start(wq_t[:, :], wq_d[:, :])
            nc.scalar.dma_start(wk_t[:, :], wk_d[:, :])
            nc.gpsimd.dma_start(wv_t[:, :], wv_d[:, :])
            nc.scalar.dma_start(wg_t[:, :], wg_d[:, :])
            nc.scalar.dma_start(wo_t[:, :], wo_d[:, :])
            nc.gpsimd.dma_start(expb_t[:, 0:2048], expb_d[:, 0:2048])
            nc.gpsimd.dma_start(expb_t[:, 2048:4096], expb_d[:, 2048:4096])
            nc.scalar.dma_start(em_t[:, :], em_d[:, :])
            nc.scalar.dma_start(bgc_t[:, :], bgc_d[:, :])
            nc.gpsimd.dma_start(id_t[:, :], id_d[:, :])

            # per-pair tiles passed from the load/proj stage to attention
            stash = {}

            def emit_load_proj(p):
                xx = wp.tile([128, 2048], f16, tag="xx")
                for s01 in range(2):
                    nc.sync.dma_start(
                        xx[:, s01 * 1024:(s01 + 1) * 1024],
                        x_d[2 * p + s01])
                x4 = xx.rearrange("p (s cc q) -> p s cc q", s=2, cc=4)

                # paired projections: rhs = [x_s0 | x_s1] per c-chunk, N=512
                def proj_T(w_t, bcc):
                    ps = pp.tile([128, 1024], f32, tag="pp")
                    for tcc in range(2):
                        for cc in range(2):
                            nc.tensor.matmul(
                                ps[:, tcc * 512:(tcc + 1) * 512].rearrange(
                                    "p (s q) -> p s q", s=2),
                                w_t[:, cc * 256 + tcc * 128:
                                    cc * 256 + tcc * 128 + 128],
                                x4[:, :, bcc + cc, :],
                                start=(cc == 0), stop=(cc == 1))
                    return ps

                qt_ps = proj_T(wq_t, 0)
                qt = wp.tile([128, 1024], f16, tag="qt")
                nc.vector.tensor_copy(qt[:, :], qt_ps[:, :])

                kt_ps = proj_T(wk_t, 2)
                kt = wp.tile([128, 1024], f16, tag="kt")
                nc.vector.tensor_copy(kt[:, :], kt_ps[:, :])

                g_ps = proj_T(wg_t, 0)
                gs0 = wp.tile([128, 1024], f16, tag="gs0")
                for tcc in range(2):
                    nc.scalar.activation(
                        gs0[:, tcc * 512:(tcc + 1) * 512],
                        g_ps[:, tcc * 512:(tcc + 1) * 512],
                        AF.Tanh, scale=0.5,
                        bias=bgc_t[:, tcc:tcc + 1])
                # gs = tanh(.)+1 so the gate apply is a plain 2-input mult
                gs = wp.tile([128, 1024], f16, tag="gs")
                nc.vector.tensor_scalar(
                    gs[:, :], gs0[:, :], 1.0, None, op0=ALU.add)

                # v natural per row; v' = v * exp(mask)[k]
                v_ps = pp.tile([128, 1024], f32, tag="pp")
                for s01 in range(2):
                    for kc in range(2):
                        for cc in range(2):
                            nc.tensor.matmul(
                                v_ps[:, s01 * 512 + kc * 256:
                                     s01 * 512 + kc * 256 + 256],
                                xx[:, s01 * 1024 + 512 + cc * 256 + kc * 128:
                                   s01 * 1024 + 512 + cc * 256 + kc * 128 + 128],
                                wv_t[:, cc * 256:(cc + 1) * 256],
                                start=(cc == 0), stop=(cc == 1))
                vs = wp.tile([128, 1024], bf16, tag="vs")
                for s01 in range(2):
                    s = 2 * p + s01
                    nc.vector.scalar_tensor_tensor(
                        vs[:, s01 * 512:(s01 + 1) * 512].rearrange(
                            "p (kc t) -> p kc t", kc=2),
                        v_ps[:, s01 * 512:(s01 + 1) * 512].rearrange(
                            "p (kc t) -> p kc t", kc=2), 1.0,
                        em_t[:, s * 64:(s + 1) * 64].rearrange(
                            "p (kc e) -> p kc e", kc=2)[:, :, 0:1
                            ].broadcast_to((128, 2, 256)),
                        op0=ALU.mult, op1=ALU.mult)
                stash[p] = (qt, kt, gs, vs)

            def emit_attention(p):
                qt, kt, gs, vs = stash.pop(p)
                ogs = []
                for s01 in range(2):
                    s = 2 * p + s01
                    og = wp4.tile([128, 512], f16, tag="og")
                    ogs.append(og)
                    ovz = pp.tile([128, 1024], f32, tag="pp")
                    ovzs = (ovz[:, 0:512], ovz[:, 512:1024])
                    for hg in range(2):
                        # per hh-pair: 2-way packed scores into a double-
                        # buffered sc pool (decouples PE from ACT).  Pair
                        # bias: hg0 is PE-seeded (identity matmul on raw
                        # pair), hg1 multiplies exp(pair) after the exp --
                        # one tile on DVE, one on GpSimd (engine balance).
                        Axs = []
                        for pr in range(2):
                            sct = pscA.tile([128, 1024], f32, tag="scA")
                            if hg < nsplit:
                                for b in range(2):
                                    nc.tensor.matmul(
                                        sct[:, b * 512:(b + 1) * 512],
                                        id_t[:, :],
                                        expb_t[:, hg * 2048 + pr * 1024 + b * 512:
                                               hg * 2048 + pr * 1024 + b * 512 + 512],
                                        start=True, stop=False,
                                        skip_group_check=True)
                            for kc in range(2):
                                for u in range(2):
                                    hh = pr * 2 + u
                                    col = u * 512 + kc * 256
                                    nc.tensor.matmul(
                                        sct[:, col:col + 256],
                                        kt[32 * hh:32 * hh + 32,
                                           hg * 512 + s01 * 256 + kc * 128:
                                           hg * 512 + s01 * 256 + kc * 128 + 128],
                                        qt[32 * hh:32 * hh + 32,
                                           hg * 512 + s01 * 256:
                                           hg * 512 + s01 * 256 + 256],
                                        start=(hg >= nsplit and kc == 0),
                                        stop=(kc == 1),
                                        tile_position=(32 * hh, 0),
                                        skip_group_check=True)
                            Ax = wp4.tile([128, 1024], bf16, tag="A")
                            if hg < nsplit:
                                nc.scalar.activation(Ax[:, :], sct[:, :], AF.Exp)
                            else:
                                eS = wp4.tile([128, 1024], bf16, tag="eS")
                                nc.scalar.activation(eS[:, :], sct[:, :], AF.Exp)
                                eng = nc.vector if (pr == 0 or not gp_mul) \
                                    else nc.gpsimd
                                eng.tensor_tensor(
                                    Ax[:, :], eS[:, :],
                                    expb_t[:, 2048 + pr * 1024:
                                           2048 + (pr + 1) * 1024],
                                    op=ALU.mult)
                            Axs.append(Ax)
                        Aa, Ab = Axs

                        for hh in range(4):
                            Ax = Aa if hh < 2 else Ab
                            for kc in range(2):
                                nc.tensor.matmul(
                                    ovzs[hg][32 * hh:32 * hh + 32, 0:256],
                                    vs[:, s01 * 512 + kc * 256 +
                                       (hg * 4 + hh) * 32:
                                       s01 * 512 + kc * 256 +
                                       (hg * 4 + hh) * 32 + 32],
                                    Ax[:, (hh % 2) * 512 + kc * 256:
                                       (hh % 2) * 512 + kc * 256 + 256],
                                    start=(kc == 0), stop=(kc == 1),
                                    tile_position=(0, 32 * hh))
                        for hh in range(4):
                            Ax = Aa if hh < 2 else Ab
                            for kc in range(2):
                                nc.tensor.matmul(
                                    ovzs[hg][32 * hh:32 * hh + 32, 256:512],
                                    em_t[:, s * 64 + kc * 32:
                                         s * 64 + kc * 32 + 32],
                                    Ax[:, (hh % 2) * 512 + kc * 256:
                                       (hh % 2) * 512 + kc * 256 + 256],
                                    start=(kc == 0), stop=(kc == 1),
                                    tile_position=(0, 32 * hh))

                        dd = wp4.tile([128, 256], f16, tag="dd")
                        if use_div:
                            nc.vector.tensor_tensor(
                                dd[:, :], ovzs[hg][:, 0:256],
                                ovzs[hg][:, 256:512],
                                op=ALU.divide)
                        else:
                            rz = wp4.tile([128, 256], f32, tag="rz")
                            nc.vector.reciprocal_approx_fast(
                                rz[:, :], ovzs[hg][:, 256:512])
                            nc.vector.tensor_tensor(
                                dd[:, :], ovzs[hg][:, 0:256],
                                rz[:, :], op=ALU.mult)
                        eng_og = nc.gpsimd if gp_mul else nc.vector
                        eng_og.tensor_tensor(
                            og[:, hg * 256:(hg + 1) * 256],
                            gs[:, hg * 512 + s01 * 256:
                               hg * 512 + s01 * 256 + 256],
                            dd[:, :], op=ALU.mult)

                # final projection (bo added on host)
                f_ps = pscA.tile([128, 1024], f32, tag="scA", name="f_ps")
                out_sb = wp.tile([128, 1024], f32, tag="out")
                for s01 in range(2):
                    og = ogs[s01]
                    for qc in range(2):
                        for tcc in range(2):
                            nc.tensor.matmul(
                                f_ps[:, s01 * 512 + qc * 256:
                                     s01 * 512 + qc * 256 + 256],
                                og[:, tcc * 256 + qc * 128:
                                   tcc * 256 + qc * 128 + 128],
                                wo_t[:, tcc * 256:(tcc + 1) * 256],
                                start=(tcc == 0), stop=(tcc == 1))
                nc.scalar.copy(out_sb[:, :], f_ps[:, :])
                for s01 in range(2):
                    nc.sync.dma_start(
                        out_d[2 * p + s01],
                        out_sb[:, s01 * 512:(s01 + 1) * 512])

            # software pipeline: proj runs KSKEW pairs ahead of attention
            kskew = int(os.environ.get("KSKEW", "1"))
            for p in range(npair + kskew):
                if p < npair:
                    emit_load_proj(p)
                if p >= kskew:
                    emit_attention(p - kskew)

    nc.compile()
    return nc


def get_program(s_loc=S_LOC):
    key = (s_loc, os.environ.get("KDIV", "recip"), os.environ.get("KGP", "0"), os.environ.get("KSPLIT", "2"), os.environ.get("KSKEW", "1"))
    if key not in _CACHE:
        _CACHE[key] = _build_program(s_loc)
    return _CACHE[key]


def prep_inputs(q_x, kv_x, bias_mask, bias_pair, Wq, Wk, Wv, Wg, bg, Wo, bo,
                s_loc=S_LOC, n_cores=N_CORES):
    """Host-side layout prep. Returns per-core in_maps."""
    bf16 = ml_dtypes.bfloat16
    f16 = np.float16

    def wprep(wt):  # (in_dim, out_dim) -> [p, (cc, out)]
        return np.ascontiguousarray(
            wt.reshape(2, 128, 256).transpose(1, 0, 2).reshape(128, 512)
        ).astype(f16)

    wq_h = wprep(np.asarray(Wq).T)     # lhsT[c, t] = Wq[t, c]
    wk_h = wprep(np.asarray(Wk).T)
    wg_h = wprep(np.asarray(Wg).T)
    wv_h = wprep(np.asarray(Wv).T)     # rhs[c, t]
    wo_h = wprep(np.asarray(Wo).T * 0.5)  # rhs[t, c]; 0.5 = sigmoid fold

    bgc = np.ascontiguousarray(
        (0.5 * np.asarray(bg, np.float32)).reshape(2, 128).T)  # [128, tc]

    # [128, (hg, pr, u, kc, q)], h = hg*4 + pr*2 + u.
    # hg0 half holds raw pair^T (PE seed); hg1 half holds exp(pair^T).
    eb = np.asarray(bias_pair[0, 0], np.float64)  # (H, Q, K)
    ebT = eb.transpose(0, 2, 1)  # (H, K, Q)
    nsplit = int(os.environ.get("KSPLIT", "2"))
    if nsplit < 2:
        ebT = np.concatenate([ebT[:4 * nsplit], np.exp(ebT[4 * nsplit:])],
                             axis=0)
    expb_h = np.ascontiguousarray(
        ebT.reshape(2, 2, 2, 2, 128, Q).transpose(4, 0, 1, 2, 3, 5
                                                  ).reshape(128, 4096)
    ).astype(bf16)

    x_all = np.concatenate([
        np.asarray(q_x[0], f16).transpose(0, 2, 1),
        np.asarray(kv_x[0], f16).transpose(0, 2, 1)], axis=1)
    # (S, 2C, Q) -> linearized [S, 128, (cc, q)] so the device DMA is
    # a plain contiguous copy (cheap SWDGE descriptors)
    x_all = np.ascontiguousarray(
        x_all.reshape(S, 4, 128, Q).transpose(0, 2, 1, 3).reshape(S, 128, 1024))
    # exp(mask) replicated 32x: [128, (s, kc, 32)]
    em_all = np.exp(np.asarray(bias_mask[0, :, 0, 0, :], np.float64))  # (S, K)

    in_maps = []
    for core in range(n_cores):
        lo = core * s_loc
        em = em_all[lo:lo + s_loc].reshape(s_loc, 2, 128)  # (s, kc, p)
        em_h = np.ascontiguousarray(np.broadcast_to(
            em.transpose(2, 0, 1)[:, :, :, None], (128, s_loc, 2, 32)
        ).reshape(128, s_loc * 64)).astype(bf16)
        in_maps.append({
            "x": x_all[lo:lo + s_loc],
            "em": em_h, "expb": expb_h,
            "wq": wq_h, "wk": wk_h, "wv": wv_h, "wg": wg_h, "wo": wo_h,
            "bgc": bgc, "ident": np.eye(128, dtype=bf16),
        })
    return in_maps


def kernel(q_x, kv_x, bias_mask, bias_pair, Wq, Wk, Wv, Wg, bg, Wo, bo):
    from concourse import bass_utils

    nc = get_program()
    in_maps = prep_inputs(q_x, kv_x, bias_mask, bias_pair,
                          Wq, Wk, Wv, Wg, bg, Wo, bo)
    res = bass_utils.run_bass_kernel_spmd(
        nc, in_maps, core_ids=list(range(N_CORES)))
    out = np.concatenate([res.results[i]["out"] for i in range(N_CORES)], axis=0)
    # undo device layout [s, p, (qc, c)] -> [s, (qc, p), c]
    out = out.reshape(S, 128, 2, C).transpose(0, 2, 1, 3).reshape(B, S, Q, C)
    return out.astype(np.float32) + np.asarray(bo, np.float32)
